# revision 23
# baseline (speedup 1.0000x reference)
"""2-layer GCN (PyG GCNConv semantics) on 8 Trainium2 NeuronCores.

Math: gcn_conv(x) = (dinv * SumAgg(xs)) @ W + b with xs = dinv * x, where
SumAgg[d] = sum_{e: dst=d} xs[src_e] + xs[d]; dinv = 1/sqrt(indeg+1). The
symmetric edge normalization dinv[src]*dinv[dst] factorizes into a row scale
of the gathered operand and a row scale of the aggregate.

Sharding: nodes dealt round-robin to 8 cores by descending degree (12500
nodes/core, padded to 12544 = 98 tiles x 128); edges partitioned by
destination owner. Weights replicated.

Layer 1: the host materializes each core's in-edge source rows x[src] in a
[dst-slot, step] grid layout (slot j of node p; j=0 is the self loop), so the
device streams them with plain sequential DMA, multiplies by per-slot
dinv[src] (from a streamed deg grid; rsqrt on device), and segment-sums with
a strided DVE reduce. Epilogue per 128-node tile: scale by dinv[dst],
transpose (PE), matmul W1 (PE), +b1/relu (ACT), transpose back, scale by
dinv -> xs2 shard rows.

Exchange: AllGather of the 12560-row xs2 shards -> full 100480-row table.

Layer 2: bulk gather of xs2[src] with dma_gather — edges sorted by source
shard, so each batch's indices are shard-relative int16 (<12560) against a
sliced table window; then dma_scatter_add (CCE inline add) accumulates each
batch into a DRAM aggregate table indexed by int16 destination slot.
Epilogue as layer 1 (W2, no trailing dinv scale) -> output shard.
"""

import sys
import types

import numpy as np

N = 100000
E = 1600000
C = 128
NCORES = 8
LOCAL = 12500
NT = 98  # dst tiles per core
LOCAL_PAD = NT * 128  # 12544
SHARD = LOCAL_PAD + 16  # 12560 (+16 zero rows; used as gather dummy target)
XS2_ROWS = SHARD * NCORES  # 100480
AGG2_ROWS = LOCAL_PAD + 64  # trash row 12544 for padded scatter edges
BATCH = 1024  # layer-2 edges per gather/scatter call
SINGLE_PACKET = True  # dma_gather/scatter_add packetization

TRACE = False
LAST_RESULTS = None


def _install_ntff_hook():
    if "antenv.axon_hooks" in sys.modules:
        return
    mod = types.ModuleType("antenv.axon_hooks")
    mod._hook = None
    mod.set_axon_ntff_profile_hook = lambda h, _m=mod: setattr(_m, "_hook", h)
    mod.get_axon_ntff_profile_hook = lambda _m=mod: _m._hook
    sys.modules["antenv.axon_hooks"] = mod
    try:
        from trn_agent_boot.trn_boot import _ntff_profile_via_ctypes

        mod.set_axon_ntff_profile_hook(
            _ntff_profile_via_ctypes("/opt/axon/libaxon_pjrt.so")
        )
    except Exception:
        pass


def _wrap16(idx_flat):
    """[n] int -> [128, n/16] int16: index i at partition i%16, slot i//16,
    replicated across the 8 gpsimd cores (16-partition groups)."""
    n = len(idx_flat)
    assert n % 16 == 0
    w = idx_flat.astype(np.int16).reshape(n // 16, 16).T  # [16, n/16]
    return np.ascontiguousarray(np.tile(w, (8, 1)))


def _preprocess(x, edge_index):
    src = edge_index[0].astype(np.int64)
    dst = edge_index[1].astype(np.int64)
    indeg = np.bincount(dst, minlength=N)
    slots = indeg + 1
    order = np.argsort(-slots, kind="stable")
    rank = np.empty(N, np.int64)
    rank[order] = np.arange(N)
    core_of = rank % NCORES
    lpos = rank // NCORES
    xs2row = core_of * SHARD + lpos

    deg_f = slots.astype(np.float32)
    deg_loc = np.ones((NCORES, LOCAL_PAD), np.float32)
    deg_loc[core_of, lpos] = deg_f
    deg_loc_pm = np.ascontiguousarray(
        deg_loc.reshape(NCORES, NT, 128).transpose(0, 2, 1)
    )  # [NC, 128, 98]

    # ---- layer-1 K-grid (slots per node incl. self loop) ----
    sl_pad = np.ones((NCORES, LOCAL_PAD), np.int64)
    sl_pad[core_of, lpos] = slots
    ktile = sl_pad.reshape(NCORES, NT, 128).max(axis=2).max(axis=0)  # [98] pooled
    ktile = np.asarray(ktile, np.int64)
    cumk = np.concatenate([[0], np.cumsum(ktile)])
    tots = int(cumk[-1])  # total steps over tiles

    ecore = core_of[dst]
    elpos = lpos[dst]
    keys = ecore * LOCAL_PAD + elpos
    perm = np.argsort(keys, kind="stable")
    ks = keys[perm]
    ssrc = src[perm]
    cnts = np.bincount(ks, minlength=NCORES * LOCAL_PAD)
    starts = np.concatenate([[0], np.cumsum(cnts)[:-1]])
    j_idx = np.arange(E) - starts[ks] + 1  # j=0 reserved for self loop
    ec = ks // LOCAL_PAD
    el = ks % LOCAL_PAD

    # grid of source node ids, -1 = dummy
    gsrc = np.full((NCORES, LOCAL_PAD, int(ktile[0])), -1, np.int64)
    ar = np.arange(N)
    gsrc[core_of, lpos, 0] = ar
    gsrc[ec, el, j_idx] = ssrc

    # materialized per-core layer-1 stream [128, tots*C]: tile t occupies
    # columns [cumk[t]*C, (cumk[t]+kt)*C) in [c][k] order (k innermost so all
    # device DVE ops are contiguous); per-slot degree grid [128, tots]
    xz = np.concatenate([x, np.zeros((1, C), np.float32)], axis=0)  # -1 -> zero row
    degz = np.concatenate([deg_f, np.ones(1, np.float32)])
    m1 = np.empty((NCORES, 128, tots * C), np.float32)
    dg1 = np.empty((NCORES, 128, tots), np.float32)
    for c in range(NCORES):
        for t in range(NT):
            kt = int(ktile[t])
            blk = gsrc[c, t * 128 : (t + 1) * 128, :kt]  # [128, kt]
            m1[c, :, cumk[t] * C : (cumk[t] + kt) * C] = (
                xz[blk].transpose(0, 2, 1).reshape(128, C * kt)
            )
            dg1[c, :, cumk[t] : cumk[t] + kt] = degz[blk]

    # ---- layer-2 batches ----
    # per core: edge list (src position in xs2_full -> window = src shard,
    # dst local slot). dma_scatter_add loses updates when two rows of one
    # call target the same dst, so within each (window, sub-batch) every dst
    # is unique: sub-batch jj = per-(window,dst) occurrence counter. Padding
    # dummies gather a zero row and scatter to a trash slot (their mutual
    # collisions there are harmless).
    # self loops are NOT in the layer-2 edge list — the epilogue adds the
    # core-local xs2 rows directly (saves traffic and keeps windows uniform)
    per_core = []  # (pos, w, d) arrays per core
    for c in range(NCORES):
        m = ec == c
        pos_a = xs2row[ssrc[m]]
        d_a = el[m]
        w_a = pos_a // SHARD
        per_core.append((pos_a % SHARD, w_a, d_a))

    # occurrence counter jj per (core, window, dst)
    subs = []  # per core: (pos, w, d, jj)
    nsub = np.zeros(NCORES, np.int64)  # per window pooled sub-batch count
    for c in range(NCORES):
        pos_a, w_a, d_a = per_core[c]
        key = w_a * LOCAL_PAD + d_a
        o = np.argsort(key, kind="stable")
        ks_ = key[o]
        cnt = np.bincount(ks_, minlength=NCORES * LOCAL_PAD)
        st = np.concatenate([[0], np.cumsum(cnt)[:-1]])
        jj = np.arange(len(ks_)) - st[ks_]
        subs.append((pos_a[o], w_a[o], d_a[o], jj))
        for w in range(NCORES):
            mm = w_a[o] == w
            if mm.any():
                nsub[w] = max(nsub[w], jj[mm].max() + 1)

    # pooled per (window, jj) counts -> padded to 128, capped at BATCH chunks
    nwj = np.zeros((NCORES, int(nsub.max())), np.int64)
    for c in range(NCORES):
        pos_a, w_a, d_a, jj = subs[c]
        for w in range(NCORES):
            mm = w_a == w
            if mm.any():
                bc = np.bincount(jj[mm])
                nwj[w, : len(bc)] = np.maximum(nwj[w, : len(bc)], bc)
    nwj = ((nwj + 127) // 128) * 128

    # call list: (window, joff, n) with n <= BATCH; layout offsets
    calls = []
    off = 0
    offs = {}
    for w in range(NCORES):
        for j in range(int(nsub[w])):
            n = int(nwj[w, j])
            if n == 0:
                continue
            offs[(w, j)] = off
            done = 0
            while done < n:
                b = min(BATCH, n - done)
                calls.append((w, off + done, b))
                done += b
            off += n
    totb = off

    g2i = np.full((NCORES, totb), SHARD - 1, np.int64)  # dummy: a zero row
    s2i = np.full((NCORES, totb), LOCAL_PAD, np.int64)  # dummy: trash slot
    for c in range(NCORES):
        pos_a, w_a, d_a, jj = subs[c]
        okey = (w_a * (int(nsub.max()) + 1) + jj).astype(np.int64)
        o = np.argsort(okey, kind="stable")
        pos_a, w_a, d_a, jj = pos_a[o], w_a[o], d_a[o], jj[o]
        # place each (w, jj) group at its offset
        grp_key = w_a * (int(nsub.max()) + 1) + jj
        uniq, starts_ = np.unique(grp_key, return_index=True)
        for ui, gk in enumerate(uniq):
            w = int(gk) // (int(nsub.max()) + 1)
            j = int(gk) % (int(nsub.max()) + 1)
            s = starts_[ui]
            e = starts_[ui + 1] if ui + 1 < len(uniq) else len(grp_key)
            g2i[c, offs[(w, j)] : offs[(w, j)] + (e - s)] = pos_a[s:e]
            s2i[c, offs[(w, j)] : offs[(w, j)] + (e - s)] = d_a[s:e]

    g2 = np.stack([_wrap16(g2i[c]) for c in range(NCORES)])  # [NC,128,totb/16]
    s2 = np.stack([_wrap16(s2i[c]) for c in range(NCORES)])

    return {
        "order": order,
        "ktile": ktile,
        "cumk": cumk,
        "tots": tots,
        "m1": m1,
        "dg1": dg1,
        "deg_loc_pm": deg_loc_pm,
        "calls": calls,
        "totb": totb,
        "g2": np.ascontiguousarray(g2),
        "s2": np.ascontiguousarray(s2),
    }


def _build_bass(ktile, cumk, tots, calls, totb):
    import concourse.bacc as bacc
    import concourse.tile as tile
    from concourse import mybir
    from concourse.masks import make_identity

    f32 = mybir.dt.float32
    i16 = mybir.dt.int16
    AF = mybir.ActivationFunctionType
    OP = mybir.AluOpType
    AX = mybir.AxisListType

    import os

    skip_l2 = os.environ.get("GCN_SKIP_L2", "0") == "1"

    nc = bacc.Bacc(
        "TRN2", target_bir_lowering=False, debug=False, num_devices=NCORES
    )

    m1_in = nc.dram_tensor("m1", [128, tots * C], f32, kind="ExternalInput")
    dg1_in = nc.dram_tensor("dg1", [128, tots], f32, kind="ExternalInput")
    deg_loc_in = nc.dram_tensor("deg_loc", [128, NT], f32, kind="ExternalInput")
    g2_in = nc.dram_tensor("g2", [128, totb // 16], i16, kind="ExternalInput")
    s2_in = nc.dram_tensor("s2", [128, totb // 16], i16, kind="ExternalInput")
    w1_in = nc.dram_tensor("w1", [C, C], f32, kind="ExternalInput")
    w2_in = nc.dram_tensor("w2", [C, C], f32, kind="ExternalInput")
    b1_in = nc.dram_tensor("b1", [C, 1], f32, kind="ExternalInput")
    b2_in = nc.dram_tensor("b2", [C, 1], f32, kind="ExternalInput")
    y_out = nc.dram_tensor("y", [LOCAL_PAD, C], f32, kind="ExternalOutput")

    with tile.TileContext(nc) as tc:
        with (
            tc.tile_pool(name="dram", bufs=1, space="DRAM") as dp,
            tc.tile_pool(name="const", bufs=1) as cp,
            tc.tile_pool(name="epi", bufs=3) as ep,
            tc.tile_pool(name="ps", bufs=2, space="PSUM") as psp,
        ):
            xs2_local = dp.tile([SHARD, C], f32)
            xs2_full = dp.tile([XS2_ROWS, C], f32, addr_space="Shared")
            agg2 = dp.tile([AGG2_ROWS, C], f32)

            ident = cp.tile([128, 128], f32)
            make_identity(nc, ident[:])
            w1_sb = cp.tile([C, C], f32)
            nc.sync.dma_start(w1_sb[:], w1_in[:])
            w2_sb = cp.tile([C, C], f32)
            nc.sync.dma_start(w2_sb[:], w2_in[:])
            b1_sb = cp.tile([C, 1], f32)
            nc.sync.dma_start(b1_sb[:], b1_in[:])
            b2_sb = cp.tile([C, 1], f32)
            nc.sync.dma_start(b2_sb[:], b2_in[:])

            # dinv tables
            dg1_sb = cp.tile([128, tots], f32)
            nc.sync.dma_start(dg1_sb[:], dg1_in[:])
            dinv1 = cp.tile([128, tots], f32)
            nc.vector.reciprocal(dinv1[:], dg1_sb[:])
            nc.scalar.activation(dinv1[:], dinv1[:], AF.Sqrt)
            deg_loc_sb = cp.tile([128, NT], f32)
            nc.sync.dma_start(deg_loc_sb[:], deg_loc_in[:])
            dinv_loc = cp.tile([128, NT], f32)
            nc.vector.reciprocal(dinv_loc[:], deg_loc_sb[:])
            nc.scalar.activation(dinv_loc[:], dinv_loc[:], AF.Sqrt)

            zt = cp.tile([128, C], f32)
            nc.vector.memset(zt[:], 0.0)
            # zero rows of xs2_local used as the layer-2 gather dummy target
            nc.sync.dma_start(xs2_local[LOCAL_PAD:SHARD, :], zt[:16, :])
            # zero agg2 before the scatter-adds
            for t in range(AGG2_ROWS // 128):
                nc.sync.dma_start(agg2[t * 128 : (t + 1) * 128, :], zt[:])
            nc.sync.dma_start(agg2[NT * 128 : AGG2_ROWS, :], zt[: AGG2_ROWS - NT * 128, :])

            def epilogue_tile(arow_ap, tt, w_sb, b_sb, layer):
                """arow_ap: [128, 128] aggregate (node-major). Writes the
                layer output rows for tile tt."""
                a_s = ep.tile([128, 128], f32, tag="a_s")
                nc.scalar.mul(a_s[:], arow_ap, dinv_loc[:, tt : tt + 1])
                p1 = psp.tile([128, 128], f32, tag="p1")
                nc.tensor.transpose(p1[:], a_s[:], ident[:])
                aT = ep.tile([128, 128], f32, tag="aT")
                nc.scalar.copy(aT[:], p1[:])
                p2 = psp.tile([128, 128], f32, tag="p2")
                nc.tensor.matmul(p2[:], lhsT=w_sb[:], rhs=aT[:], start=True, stop=True)
                xt_ = ep.tile([128, 128], f32, tag="xt_")
                nc.scalar.activation(xt_[:], p2[:], AF.Relu, bias=b_sb[:])
                p3 = psp.tile([128, 128], f32, tag="p3")
                nc.tensor.transpose(p3[:], xt_[:], ident[:])
                o_t = ep.tile([128, 128], f32, tag="o_t")
                if layer == 1:
                    nc.scalar.mul(o_t[:], p3[:], dinv_loc[:, tt : tt + 1])
                    nc.sync.dma_start(
                        xs2_local[tt * 128 : (tt + 1) * 128, :], o_t[:]
                    )
                else:
                    nc.scalar.copy(o_t[:], p3[:])
                    nc.sync.dma_start(y_out[tt * 128 : (tt + 1) * 128, :], o_t[:])

            # ---- layer 1: stream materialized grid, scale, reduce ----
            # tile layout [128, C, kt] (k innermost) so multiply and reduce
            # are fully contiguous on DVE
            with tc.tile_pool(name="l1", bufs=4) as lp:
                for t in range(NT):
                    kt = int(ktile[t])
                    c0 = int(cumk[t])
                    mt = lp.tile([128, C, kt], f32, tag="mt")
                    nc.sync.dma_start(
                        mt[:, :, :].rearrange("p c k -> p (c k)"),
                        m1_in[:, c0 * C : (c0 + kt) * C],
                    )
                    dv = (
                        dinv1[:, c0 : c0 + kt]
                        .rearrange("p (o k) -> p o k", o=1)
                        .to_broadcast([128, C, kt])
                    )
                    ms = lp.tile([128, C, kt], f32, tag="ms")
                    nc.vector.tensor_tensor(
                        out=ms[:, :, :], in0=mt[:, :, :], in1=dv, op=OP.mult
                    )
                    arow = lp.tile([128, C], f32, tag="arow")
                    nc.vector.tensor_reduce(
                        out=arow[:, :],
                        in_=ms[:, :, :],
                        axis=AX.X,
                        op=OP.add,
                    )
                    epilogue_tile(arow[:, :], t, w1_sb, b1_sb, layer=1)

            if skip_l2:
                # bisect mode: emit the layer-1 result (xs2 shard) as y
                with tc.tile_pool(name="dbg", bufs=3) as dbp:
                    for t in range(NT):
                        dt_ = dbp.tile([128, C], f32, tag="dt_")
                        nc.sync.dma_start(
                            dt_[:, :], xs2_local[t * 128 : (t + 1) * 128, :]
                        )
                        nc.sync.dma_start(
                            y_out[t * 128 : (t + 1) * 128, :], dt_[:, :]
                        )

            # ---- exchange ----
            if not skip_l2:
                nc.gpsimd.collective_compute(
                    "AllGather",
                    mybir.AluOpType.bypass,
                    replica_groups=[list(range(NCORES))],
                    ins=[xs2_local[:, :]],
                    outs=[xs2_full[:, :]],
                )

            # ---- layer 2: windowed bulk gather + collision-free scatter-add ----
            with tc.tile_pool(name="l2", bufs=6) as lp2:
                for w, boff, b in ([] if skip_l2 else calls):
                    win = xs2_full[w * SHARD : (w + 1) * SHARD, :]
                    gidx = lp2.tile(
                        [128, b // 16], i16, tag="gidx",
                    )
                    nc.sync.dma_start(
                        gidx[:, :], g2_in[:, boff // 16 : (boff + b) // 16]
                    )
                    sidx = lp2.tile(
                        [128, b // 16], i16, tag="sidx",
                    )
                    nc.sync.dma_start(
                        sidx[:, :], s2_in[:, boff // 16 : (boff + b) // 16]
                    )
                    msg = lp2.tile(
                        [128, b // 128, C], f32, tag="msg",
                    )
                    nc.gpsimd.dma_gather(
                        out_ap=msg[:, :, :],
                        in_ap=win,
                        idxs_ap=gidx[:, :],
                        num_idxs=b,
                        num_idxs_reg=b,
                        elem_size=C,
                        single_packet=SINGLE_PACKET,
                    )
                    nc.gpsimd.dma_scatter_add(
                        out_ap=agg2[:, :],
                        in_ap=msg[:, :, :],
                        idxs_ap=sidx[:, :],
                        num_idxs=b,
                        num_idxs_reg=b,
                        elem_size=C,
                        single_packet=SINGLE_PACKET,
                    )

                for t in range(NT) if not skip_l2 else []:
                    ar2 = lp2.tile([128, C], f32, tag="ar2")
                    nc.sync.dma_start(ar2[:, :], agg2[t * 128 : (t + 1) * 128, :])
                    xsl = lp2.tile([128, C], f32, tag="xsl")
                    nc.sync.dma_start(
                        xsl[:, :], xs2_local[t * 128 : (t + 1) * 128, :]
                    )
                    ar2s = lp2.tile([128, C], f32, tag="ar2s")
                    # self-loop contribution: + own xs2 rows
                    nc.vector.tensor_tensor(
                        out=ar2s[:, :], in0=ar2[:, :], in1=xsl[:, :], op=OP.add
                    )
                    epilogue_tile(ar2s[:, :], t, w2_sb, b2_sb, layer=2)

    nc.compile()
    return nc


def kernel(x, edge_index, W1, b1, W2, b2):
    global LAST_RESULTS
    _install_ntff_hook()
    from concourse import bass_utils

    bass_utils.upload_artifacts = lambda tmpdir: ""

    x = np.asarray(x, dtype=np.float32)
    edge_index = np.asarray(edge_index, dtype=np.int32)
    W1 = np.asarray(W1, dtype=np.float32)
    b1 = np.asarray(b1, dtype=np.float32)
    W2 = np.asarray(W2, dtype=np.float32)
    b2 = np.asarray(b2, dtype=np.float32)

    pre = _preprocess(x, edge_index)
    nc = _build_bass(
        pre["ktile"], pre["cumk"], pre["tots"], pre["calls"], pre["totb"]
    )

    in_maps = []
    for c in range(NCORES):
        in_maps.append(
            {
                "m1": pre["m1"][c],
                "dg1": pre["dg1"][c],
                "deg_loc": pre["deg_loc_pm"][c],
                "g2": pre["g2"][c],
                "s2": pre["s2"][c],
                "w1": W1,
                "w2": W2,
                "b1": b1.reshape(C, 1),
                "b2": b2.reshape(C, 1),
            }
        )

    res = bass_utils.run_bass_kernel_spmd(
        nc, in_maps, core_ids=list(range(NCORES)), trace=TRACE
    )
    LAST_RESULTS = res

    order = pre["order"]
    y_full = np.empty((N, C), np.float32)
    for c in range(NCORES):
        y_full[order[c::NCORES]] = res.results[c]["y"][:LOCAL]
    return y_full
